# revision 15
# baseline (speedup 1.0000x reference)
"""Self-contained Trainium2 Bass kernel for causal multi-head attention.

Problem: y = Attention(x) with B=2, T=2048, C=1024, H=16 heads, HD=64,
causal softmax, fused qkv projection and output projection.

Sharding (8 NeuronCores): core c owns heads {2c, 2c+1} for BOTH batches
(tensor parallel on heads; the batch dim rides along inside each core as
two attention "pairs"). All compute runs in bf16 (fp32 PSUM accumulation):

  per token-block qb (512 tokens, 4 blocks):
    qkv projection for both batches (q/k transposed: head-dim on
    partitions; v natural; softmax normalizer Z rides as a 65th ones
    column on v)
    causal-softmax attention for batch 0 then batch 1 (two heads
    row-tiled concurrently on the PE)
    normalized y^T (bf16) is flushed to DRAM and exchanged with a
    single 8-way AllToAll: shard j of block qb = y^T[my 2 heads,
    tokens of (batch j//4, stripe j%4)] -> after the exchange each
    core holds ALL 16 heads for its own 128-token stripe of qb
    out-projection for block qb-1 (full 1024 columns, contraction over
    the 8 gathered head-chunks) overlaps the next block's attention

The AllToAll moves only ~0.25 MB/core total (vs 6 MB for a y AllGather),
so all communication hides behind attention compute. The v-bias is folded
into an effective output bias on the host (softmax rows sum to 1).
"""

import sys

sys.path.insert(0, "/opt/trn_rl_repo")

import numpy as np

B, T, C, H, HD = 2, 2048, 1024, 16, 64
P = 128
N_CORES = 8
HPC = 2  # heads per core
QB = 512  # query block (free dim of attention matmuls)
NQB = T // QB  # 4
NCC = C // P  # 8 contraction chunks
NTT = T // P  # 16 token tiles

_cache: dict = {}


def _build_program(repeat: int = 1, single: bool = False, dbg: bool = False):
    """Build + compile the per-core Bass program (same program on all cores)."""
    import concourse.bass as bass
    import concourse.mybir as mybir
    from concourse import bacc, tile

    f32 = mybir.dt.float32
    f32r = mybir.dt.float32r
    bf16 = mybir.dt.bfloat16
    Exp = mybir.ActivationFunctionType.Exp
    Ident = mybir.ActivationFunctionType.Identity
    mult = mybir.AluOpType.mult
    add = mybir.AluOpType.add

    nc = bacc.Bacc("TRN2", target_bir_lowering=False, debug=False,
                   num_devices=N_CORES)

    xt0 = nc.dram_tensor("xt0", [C, T], bf16, kind="ExternalInput").ap()
    xt1 = nc.dram_tensor("xt1", [C, T], bf16, kind="ExternalInput").ap()
    wq = nc.dram_tensor("wq", [C, P], bf16, kind="ExternalInput").ap()
    wk = nc.dram_tensor("wk", [C, P], bf16, kind="ExternalInput").ap()
    wv = nc.dram_tensor("wv", [C, P], bf16, kind="ExternalInput").ap()
    wout = nc.dram_tensor("wout", [C, C], bf16, kind="ExternalInput").ap()
    bqk = nc.dram_tensor("bqk", [2, P], f32, kind="ExternalInput").ap()
    bout = nc.dram_tensor("bout", [1, C], f32, kind="ExternalInput").ap()
    out = nc.dram_tensor("out", [NQB, P, C], f32, kind="ExternalOutput").ap()
    if dbg:
        ydbg = nc.dram_tensor("ydbg", [B, P, T], bf16,
                              kind="ExternalOutput").ap()
        gdbg = nc.dram_tensor("gdbg", [NQB, N_CORES, P, P], bf16,
                              kind="ExternalOutput").ap()

    xt_r = [xt0.rearrange("(o p) t -> p o t", p=P),
            xt1.rearrange("(o p) t -> p o t", p=P)]

    with tile.TileContext(nc) as tc:
        import contextlib

        with contextlib.ExitStack() as ctx:
            const = ctx.enter_context(tc.tile_pool(name="const", bufs=1))
            wpool = ctx.enter_context(tc.tile_pool(name="wpool", bufs=1))
            xpool = ctx.enter_context(tc.tile_pool(name="xpool", bufs=2))
            slab = ctx.enter_context(tc.tile_pool(name="slab", bufs=1))
            work = ctx.enter_context(tc.tile_pool(name="work", bufs=3))
            psum = ctx.enter_context(tc.tile_pool(name="psum", bufs=2,
                                                  space="PSUM"))
            dram = ctx.enter_context(tc.tile_pool(name="dram", bufs=1,
                                                  space="DRAM"))

            # ---- constants ----
            ones1_64 = const.tile([1, 64], bf16)
            nc.vector.memset(ones1_64[:], 1.0)
            ones1 = const.tile([1, P], bf16)
            nc.vector.memset(ones1[:], 1.0)
            # causal mask strip: maskS[kp, u] = 1.0 iff u - kp - 384 >= 0.
            # chunk (qb, kc) with off = kc*128 - qb*512 in {0,128,256,384}
            # uses slice maskS[:, 384-off+c] for chunk column c.
            maskF = const.tile([P, 896], f32)
            nc.vector.memset(maskF[:], 1.0)
            nc.gpsimd.affine_select(
                out=maskF[:],
                in_=maskF[:],
                compare_op=mybir.AluOpType.is_ge,
                fill=0.0,
                base=-384,
                pattern=[[1, 896]],
                channel_multiplier=-1,
            )
            maskS = const.tile([P, 896], bf16)
            nc.vector.tensor_copy(out=maskS[:], in_=maskF[:])
            bqk_sb = const.tile([P, 2], f32)
            bout_row = const.tile([1, C], f32)
            bias_rep = const.tile([P, 2, QB], f32)

            # ---- weights (w_out DMA is issued later, behind the first x) ----
            wq_sb = wpool.tile([P, NCC, P], bf16, name="w_q")
            wk_sb = wpool.tile([P, NCC, P], bf16, name="w_k")
            wv_sb = wpool.tile([P, NCC, P], bf16, name="w_v")
            wout_sb = wpool.tile([P, NCC, C], bf16, name="w_out")
            nc.sync.dma_start(wq_sb[:], wq.rearrange("(o p) m -> p o m", p=P))
            nc.sync.dma_start(bqk_sb[:], bqk.rearrange("g p -> p g"))

            # PE warm-up: dependency-free matmuls on constants run during
            # the initial DMA-only window so the HAM clock gate reaches
            # full rate before the first real matmul
            warm = psum.tile([64, P], f32, tag="yt", bufs=4, name="warm")
            for _ in range(32):
                nc.tensor.matmul(warm[:], ones1_64[:], ones1[:],
                                 start=True, stop=True,
                                 skip_group_check=True)

            # ---- persistent slabs (index = batch) ----
            qt = [slab.tile([P, T], bf16, name=f"qt{b}") for b in range(B)]
            kt = [slab.tile([P, T], bf16, name=f"kt{b}") for b in range(B)]
            # v slab: per token-tile, 2 head slots of [64 v-dims | 1.0]
            # (the ones column folds the softmax normalizer Z into attn @ v)
            vsl = [slab.tile([P, NTT, HPC, HD + 1], bf16, name=f"vsl{b}")
                   for b in range(B)]
            for b in range(B):
                nc.vector.memset(
                    vsl[b].rearrange("p t h x -> p (t h) x")[:, :, HD:HD + 1],
                    1.0)
            ytsb = [slab.tile([P, T], bf16, name=f"ytsb{b}") for b in range(B)]

            for rep in range(repeat):
                if rep == 0:
                    first_extra = [
                        (wk_sb, wk), (wv_sb, wv), (wout_sb, wout)]
                else:
                    first_extra = []

                def proj(qb, rep=rep, extra=None):
                    qsl = slice(qb * QB, (qb + 1) * QB)
                    xc = [xpool.tile([P, NCC, QB], bf16, tag=f"xc{b}",
                                     name=f"xc{b}_{rep}_{qb}")
                          for b in range(B)]
                    for o in range(NCC):  # split so compute starts early
                        nc.sync.dma_start(xc[0][:, o, :], xt_r[0][:, o, qsl])
                    if extra:
                        for sb, dr in extra:
                            nc.sync.dma_start(
                                sb[:], dr.rearrange("(o p) m -> p o m", p=P))
                        nc.sync.dma_start(bout_row[:], bout)
                        nc.gpsimd.partition_broadcast(
                            bias_rep.rearrange("p h x -> p (h x)"),
                            bout_row[:], channels=P)
                    for o in range(NCC):
                        nc.sync.dma_start(xc[1][:, o, :], xt_r[1][:, o, qsl])
                    # batch-0 q/k first so its attention unblocks early;
                    # slab writes via ACT (idle-ish here) so they aren't
                    # queued behind DVE mask/normalize work
                    proj_groups = [
                        (wq_sb, 0, 0, qt[0]), (wk_sb, 1, 0, kt[0]),
                        (wq_sb, 0, 1, qt[1]), (wk_sb, 1, 1, kt[1]),
                    ]
                    for gi, (wsb, bi, b, dest) in enumerate(proj_groups[:2]):
                        ps = psum.tile([P, QB], f32, tag="a",
                                       name=f"proj_{rep}_{qb}_{gi}")
                        for o in range(NCC):
                            nc.tensor.matmul(
                                ps[:], wsb[:, o, :], xc[b][:, o, :],
                                start=(o == 0), stop=(o == NCC - 1),
                            )
                        nc.scalar.activation(dest[:, qsl], ps[:], Ident,
                                             bias=bqk_sb[:, bi:bi + 1])
                    for b in range(B):
                        for tt in range(QB // P):
                            t0 = qb * (QB // P) + tt
                            pv = psum.tile([P, P], f32, tag="yt", bufs=4,
                                           name=f"pv_{rep}_{b}_{t0}")
                            for o in range(NCC):
                                nc.tensor.matmul(
                                    pv[:], xc[b][:, o, tt * P:(tt + 1) * P],
                                    wv_sb[:, o, :],
                                    start=(o == 0), stop=(o == NCC - 1),
                                )
                            nc.vector.tensor_copy(
                                out=vsl[b][:, t0, :, 0:HD],
                                in_=pv.rearrange("p (h x) -> p h x", h=HPC))
                    for gi, (wsb, bi, b, dest) in enumerate(proj_groups[2:]):
                        ps = psum.tile([P, QB], f32, tag="a",
                                       name=f"proj_{rep}_{qb}_{2 + gi}")
                        for o in range(NCC):
                            nc.tensor.matmul(
                                ps[:], wsb[:, o, :], xc[b][:, o, :],
                                start=(o == 0), stop=(o == NCC - 1),
                            )
                        nc.vector.tensor_scalar_add(dest[:, qsl], ps[:],
                                                    bqk_sb[:, bi:bi + 1])

                def attn(b, qb, rep=rep):
                    qsl0 = qb * QB
                    ytp = [psum.tile([P, QB], f32, tag="yt", bufs=4,
                                     name=f"yt_{rep}_{b}_{qb}_{h}")
                           for h in range(2)]
                    nkc = (qb + 1) * (QB // P)
                    for kc in range(nkc):
                        off = kc * P - qb * QB
                        diag = off >= 0
                        es = off if diag else 0
                        sc = psum.tile([P, 2, QB], f32, tag="a",
                                       name=f"sc_{rep}_{b}_{qb}_{kc}")
                        for h in range(2):
                            hp = slice(h * 64, (h + 1) * 64)
                            nc.tensor.matmul(
                                sc[:, h, es:],
                                kt[b][hp, kc * P:(kc + 1) * P],
                                qt[b][hp, qsl0 + es:qsl0 + QB],
                                start=True, stop=True,
                                tile_position=(h * 64, 0),
                                skip_group_check=True,
                            )
                        et = work.tile([P, 2, QB], bf16, tag="et", bufs=4,
                                       name=f"et_{rep}_{b}_{qb}_{kc}")
                        nc.scalar.activation(et[:, :, es:], sc[:, :, es:],
                                             Exp, scale=0.125)
                        if diag:  # zero the non-causal band
                            be = min(off + P, QB)
                            for h in range(2):
                                nc.vector.tensor_tensor(
                                    et[:, h, es:be], et[:, h, es:be],
                                    maskS[:, 384 - off + es:384 - off + be],
                                    mult)
                        first, last = kc == 0, kc == nkc - 1
                        for h in range(2):
                            # [v | 1] lhsT: row 64 of the output is Z
                            nc.tensor.matmul(
                                ytp[h][0:HD + 1, es:],
                                vsl[b][:, kc, h, :],
                                et[:, h, es:],
                                start=first, stop=last,
                                skip_group_check=True,
                            )
                    for h in range(2):
                        # 1/Z replicated across the 64 head dims via a K=1
                        # ones-matmul — NOT gpsimd.partition_broadcast: the
                        # collective_compute instruction blocks the gpsimd
                        # queue until the ring completes, which would stall
                        # every later broadcast behind an AllToAll
                        zi = work.tile([1, QB], bf16, tag="zi",
                                       name=f"zi_{rep}_{b}_{qb}_{h}")
                        with nc.allow_low_precision(
                                reason="bf16 1/Z feeds replicate matmul"):
                            nc.vector.reciprocal(zi[:], ytp[h][HD:HD + 1, :])
                        zr = psum.tile([HD, QB], f32, tag="yt", bufs=4,
                                       name=f"zr_{rep}_{b}_{qb}_{h}")
                        nc.tensor.matmul(zr[:], ones1_64[:], zi[:],
                                         start=True, stop=True,
                                         skip_group_check=True)
                        zrs = work.tile([HD, QB], f32, tag="zrs",
                                        name=f"zrs_{rep}_{b}_{qb}_{h}")
                        nc.vector.tensor_copy(out=zrs[:], in_=zr[:])
                        nc.vector.tensor_tensor(
                            ytsb[b][h * HD:(h + 1) * HD, qsl0:qsl0 + QB],
                            ytp[h][0:HD, :], zrs[:], mult)

                # out-projection for one gathered token stripe: full 1024
                # columns, contraction over the 8 head-chunks (one per rank)
                def outproj(qb, ytg, rep=rep):
                    ytt = work.tile([P, N_CORES, P], bf16, tag="ytt", bufs=4,
                                    name=f"ytt_{rep}_{qb}")
                    nc.sync.dma_start(ytt[:],
                                      ytg.rearrange("i h t -> h i t"))
                    osb = work.tile([P, 2, QB], f32, tag="osb", bufs=2,
                                    name=f"osb_{rep}_{qb}")
                    for half in range(2):
                        hsl = slice(half * QB, (half + 1) * QB)
                        po = psum.tile([P, QB], f32, tag="a",
                                       name=f"po_{rep}_{qb}_{half}")
                        for i in range(N_CORES):
                            nc.tensor.matmul(
                                po[:], ytt[:, i, :], wout_sb[:, i, hsl],
                                start=(i == 0), stop=(i == N_CORES - 1),
                                skip_group_check=True,
                            )
                        nc.vector.tensor_tensor(osb[:, half, :], po[:],
                                                bias_rep[:, half, :], add)
                    nc.sync.dma_start(
                        out[qb].rearrange("p (h x) -> p h x", h=2), osb[:])

                # phase A: all projections back-to-back (dense PE, HAM warm)
                for qb in range(NQB):
                    proj(qb, extra=(first_extra if qb == 0 else None))
                # phase B: attention in DESCENDING block order — the big
                # causal blocks finish (and start their AllToAll) early,
                # and the accumulated out-projections cover the small
                # final block's exchange latency
                ytgs = {}
                order = list(range(NQB - 1, -1, -1))
                for oi, qb in enumerate(order):
                    qsl = slice(qb * QB, (qb + 1) * QB)
                    attn(0, qb)
                    attn(1, qb)
                    # flush y^T (bf16) and exchange: shard j = my 2 heads,
                    # tokens of (batch j//4, stripe j%4) within this block
                    ytl = dram.tile([N_CORES, P, P], bf16,
                                    name=f"ytl_{rep}_{qb}")
                    for b in range(B):
                        nc.sync.dma_start(
                            ytl[b * 4:(b + 1) * 4].rearrange("j h t -> h j t"),
                            ytsb[b][:, qsl].rearrange("h (j t) -> h j t", j=4))
                    ytg = dram.tile([N_CORES, P, P], bf16,
                                    name=f"ytg_{rep}_{qb}")
                    if single:
                        for g in range(N_CORES):  # timing stand-in
                            nc.gpsimd.dma_start(ytg[g], ytl[g])
                    else:
                        nc.gpsimd.collective_compute(
                            "AllToAll",
                            mybir.AluOpType.bypass,
                            replica_groups=[list(range(N_CORES))],
                            ins=[ytl.opt()],
                            outs=[ytg.opt()],
                        )
                    ytgs[qb] = ytg
                    if dbg and rep == 0:
                        for b in range(B):
                            nc.sync.dma_start(ydbg[b][:, qsl],
                                              ytsb[b][:, qsl])
                # all out-projections run at the tail: blocks 3..1 exchanged
                # long ago, so their matmuls fill the PE while block 0's
                # AllToAll completes
                for qb in order:
                    outproj(qb, ytgs[qb])
                if dbg and rep == 0:
                    for qb in range(NQB):
                        gsb = work.tile([P, N_CORES, P], bf16, tag="ytt",
                                        bufs=2, name=f"gsb_{qb}")
                        nc.sync.dma_start(
                            gsb[:], ytgs[qb].rearrange("i h t -> h i t"))
                        nc.sync.dma_start(
                            gdbg[qb].rearrange("i h t -> h i t"), gsb[:])

    nc.compile()
    return nc


def _get_program(repeat: int = 1, single: bool = False, dbg: bool = False):
    key = ("nc", repeat, single, dbg)
    if key not in _cache:
        _cache[key] = _build_program(repeat, single, dbg)
    return _cache[key]


def prepare_in_maps(x, w_qkv, b_qkv, w_out, b_out):
    """Shard full inputs into the 8 per-core input maps."""
    import concourse.mybir as mybir

    np_bf16 = mybir.dt.np(mybir.dt.bfloat16)

    x = np.asarray(x, dtype=np.float32)
    w_qkv = np.asarray(w_qkv, dtype=np.float32)
    b_qkv = np.asarray(b_qkv, dtype=np.float32)
    w_out = np.asarray(w_out, dtype=np.float32)
    b_out = np.asarray(b_out, dtype=np.float32)

    xts = [np.ascontiguousarray(x[b].T).astype(np_bf16) for b in range(B)]
    # softmax rows sum to 1 => y = attn @ v + b_v exactly, so the v-bias
    # folds into an effective output bias on the host
    b_out_eff = (b_out.astype(np.float64)
                 + b_qkv[2 * C:].astype(np.float64) @ w_out.astype(np.float64)
                 ).astype(np.float32)
    wout_bf = np.ascontiguousarray(w_out).astype(np_bf16)

    in_maps = []
    for c in range(N_CORES):
        qc = slice(c * HPC * HD, (c + 1) * HPC * HD)  # 128 cols, my 2 heads
        in_maps.append({
            "xt0": xts[0],
            "xt1": xts[1],
            "wq": np.ascontiguousarray(w_qkv[:, qc]).astype(np_bf16),
            "wk": np.ascontiguousarray(
                w_qkv[:, C + qc.start:C + qc.stop]).astype(np_bf16),
            "wv": np.ascontiguousarray(
                w_qkv[:, 2 * C + qc.start:2 * C + qc.stop]).astype(np_bf16),
            "wout": wout_bf,
            "bqk": np.ascontiguousarray(
                np.stack([b_qkv[qc], b_qkv[C + qc.start:C + qc.stop]])),
            "bout": np.ascontiguousarray(b_out_eff[None, :]),
        })
    return in_maps


def run_device(in_maps, repeat: int = 1):
    """Execute the compiled SPMD program; returns per-core result dicts.

    The NeuronCores occasionally come up wedged (NRT_EXEC_UNIT_UNRECOVERABLE
    / LoadExecutable failures) if a previous process died mid-execution;
    they recover after a short wait, so retry with backoff.
    """
    import time as _time
    from concourse import bass_utils

    nc = _get_program(repeat)
    last_err = None
    for attempt in range(3):
        try:
            res = bass_utils.run_bass_kernel_spmd(
                nc, in_maps, core_ids=list(range(N_CORES)))
            return res.results
        except Exception as e:  # device wedge: wait for recovery and retry
            last_err = e
            if attempt < 2:
                _time.sleep(75)
    raise last_err


def assemble_output(results):
    out = np.empty((B, T, C), dtype=np.float32)
    for c in range(N_CORES):
        b, j = divmod(c, 4)
        o = results[c]["out"]  # [NQB, 128, C]
        for qb in range(NQB):
            t0 = qb * QB + j * P
            out[b, t0:t0 + P, :] = o[qb]
    return out


def kernel(x, w_qkv, b_qkv, w_out, b_out):
    in_maps = prepare_in_maps(x, w_qkv, b_qkv, w_out, b_out)
    results = run_device(in_maps)
    return assemble_output(results)


if __name__ == "__main__":
    rng = np.random.default_rng(0)
    inputs = {
        "x": rng.standard_normal((B, T, C), dtype=np.float32),
        "w_qkv": rng.standard_normal((C, 3 * C), dtype=np.float32) / np.sqrt(C),
        "b_qkv": rng.standard_normal(3 * C, dtype=np.float32) * 0.1,
        "w_out": rng.standard_normal((C, C), dtype=np.float32) / np.sqrt(C),
        "b_out": rng.standard_normal(C, dtype=np.float32) * 0.1,
    }
    y = kernel(**inputs)
    print("kernel output:", y.shape, y.dtype, float(np.abs(y).max()))


# revision 16
# speedup vs baseline: 1.0700x; 1.0700x over previous
"""Self-contained Trainium2 Bass kernel for causal multi-head attention.

Problem: y = Attention(x) with B=2, T=2048, C=1024, H=16 heads, HD=64,
causal softmax, fused qkv projection and output projection.

Sharding (8 NeuronCores): core c owns heads {2c, 2c+1} for BOTH batches
(tensor parallel on heads; the batch dim rides along inside each core as
two attention "pairs"). All compute runs in bf16 (fp32 PSUM accumulation):

  per token-block qb (512 tokens, 4 blocks):
    qkv projection for both batches (q/k transposed: head-dim on
    partitions; v natural; softmax normalizer Z rides as a 65th ones
    column on v)
    causal-softmax attention for batch 0 then batch 1 (two heads
    row-tiled concurrently on the PE)
    normalized y^T (bf16) is flushed to DRAM and exchanged with a
    single 8-way AllToAll: shard j of block qb = y^T[my 2 heads,
    tokens of (batch j//4, stripe j%4)] -> after the exchange each
    core holds ALL 16 heads for its own 128-token stripe of qb
    out-projection for block qb-1 (full 1024 columns, contraction over
    the 8 gathered head-chunks) overlaps the next block's attention

The AllToAll moves only ~0.25 MB/core total (vs 6 MB for a y AllGather),
so all communication hides behind attention compute. The v-bias is folded
into an effective output bias on the host (softmax rows sum to 1).
"""

import sys

sys.path.insert(0, "/opt/trn_rl_repo")

import numpy as np

B, T, C, H, HD = 2, 2048, 1024, 16, 64
P = 128
N_CORES = 8
HPC = 2  # heads per core
QB = 512  # query block (free dim of attention matmuls)
NQB = T // QB  # 4
NCC = C // P  # 8 contraction chunks
NTT = T // P  # 16 token tiles

_cache: dict = {}


def _build_program(repeat: int = 1, single: bool = False, dbg: bool = False):
    """Build + compile the per-core Bass program (same program on all cores)."""
    import concourse.bass as bass
    import concourse.mybir as mybir
    from concourse import bacc, tile

    f32 = mybir.dt.float32
    f32r = mybir.dt.float32r
    bf16 = mybir.dt.bfloat16
    Exp = mybir.ActivationFunctionType.Exp
    Ident = mybir.ActivationFunctionType.Identity
    mult = mybir.AluOpType.mult
    add = mybir.AluOpType.add

    nc = bacc.Bacc("TRN2", target_bir_lowering=False, debug=False,
                   num_devices=N_CORES)

    xt0 = nc.dram_tensor("xt0", [C, T], bf16, kind="ExternalInput").ap()
    xt1 = nc.dram_tensor("xt1", [C, T], bf16, kind="ExternalInput").ap()
    wq = nc.dram_tensor("wq", [C, P], bf16, kind="ExternalInput").ap()
    wk = nc.dram_tensor("wk", [C, P], bf16, kind="ExternalInput").ap()
    wv = nc.dram_tensor("wv", [C, P], bf16, kind="ExternalInput").ap()
    wout = nc.dram_tensor("wout", [C, C], bf16, kind="ExternalInput").ap()
    bqk = nc.dram_tensor("bqk", [2, P], f32, kind="ExternalInput").ap()
    bout = nc.dram_tensor("bout", [1, C], f32, kind="ExternalInput").ap()
    out = nc.dram_tensor("out", [NQB, P, C], f32, kind="ExternalOutput").ap()
    if dbg:
        ydbg = nc.dram_tensor("ydbg", [B, P, T], bf16,
                              kind="ExternalOutput").ap()
        gdbg = nc.dram_tensor("gdbg", [NQB, N_CORES, P, P], bf16,
                              kind="ExternalOutput").ap()

    xt_r = [xt0.rearrange("(o p) t -> p o t", p=P),
            xt1.rearrange("(o p) t -> p o t", p=P)]

    with tile.TileContext(nc) as tc:
        import contextlib

        with contextlib.ExitStack() as ctx:
            const = ctx.enter_context(tc.tile_pool(name="const", bufs=1))
            wpool = ctx.enter_context(tc.tile_pool(name="wpool", bufs=1))
            xpool = ctx.enter_context(tc.tile_pool(name="xpool", bufs=2))
            slab = ctx.enter_context(tc.tile_pool(name="slab", bufs=1))
            work = ctx.enter_context(tc.tile_pool(name="work", bufs=3))
            psum = ctx.enter_context(tc.tile_pool(name="psum", bufs=2,
                                                  space="PSUM"))
            dram = ctx.enter_context(tc.tile_pool(name="dram", bufs=1,
                                                  space="DRAM"))

            # ---- constants ----
            ones1_64 = const.tile([1, 64], bf16)
            nc.vector.memset(ones1_64[:], 1.0)
            ones1 = const.tile([1, P], bf16)
            nc.vector.memset(ones1[:], 1.0)
            # causal mask strip: maskS[kp, u] = 1.0 iff u - kp - 384 >= 0.
            # chunk (qb, kc) with off = kc*128 - qb*512 in {0,128,256,384}
            # uses slice maskS[:, 384-off+c] for chunk column c.
            maskF = const.tile([P, 896], f32)
            nc.vector.memset(maskF[:], 1.0)
            nc.gpsimd.affine_select(
                out=maskF[:],
                in_=maskF[:],
                compare_op=mybir.AluOpType.is_ge,
                fill=0.0,
                base=-384,
                pattern=[[1, 896]],
                channel_multiplier=-1,
            )
            maskS = const.tile([P, 896], bf16)
            nc.vector.tensor_copy(out=maskS[:], in_=maskF[:])
            bqk_sb = const.tile([P, 2], f32)
            bout_row = const.tile([1, C], f32)
            bias_rep = const.tile([P, 2, QB], f32)

            # ---- weights (w_out DMA is issued later, behind the first x) ----
            wq_sb = wpool.tile([P, NCC, P], bf16, name="w_q")
            wk_sb = wpool.tile([P, NCC, P], bf16, name="w_k")
            wv_sb = wpool.tile([P, NCC, P], bf16, name="w_v")
            wout_sb = wpool.tile([P, NCC, C], bf16, name="w_out")
            nc.sync.dma_start(wq_sb[:], wq.rearrange("(o p) m -> p o m", p=P))
            nc.sync.dma_start(bqk_sb[:], bqk.rearrange("g p -> p g"))

            # PE warm-up: dependency-free matmuls on constants run during
            # the initial DMA-only window so the HAM clock gate reaches
            # full rate before the first real matmul
            warm = psum.tile([64, P], f32, tag="yt", bufs=4, name="warm")
            for _ in range(32):
                nc.tensor.matmul(warm[:], ones1_64[:], ones1[:],
                                 start=True, stop=True,
                                 skip_group_check=True)

            # ---- persistent slabs (index = batch) ----
            qt = [slab.tile([P, T], bf16, name=f"qt{b}") for b in range(B)]
            kt = [slab.tile([P, T], bf16, name=f"kt{b}") for b in range(B)]
            # v slab: per token-tile, 2 head slots of [64 v-dims | 1.0]
            # (the ones column folds the softmax normalizer Z into attn @ v)
            vsl = [slab.tile([P, NTT, HPC, HD + 1], bf16, name=f"vsl{b}")
                   for b in range(B)]
            for b in range(B):
                nc.vector.memset(
                    vsl[b].rearrange("p t h x -> p (t h) x")[:, :, HD:HD + 1],
                    1.0)
            ytsb = [slab.tile([P, T], bf16, name=f"ytsb{b}") for b in range(B)]

            for rep in range(repeat):
                if rep == 0:
                    first_extra = [
                        (wk_sb, wk), (wv_sb, wv), (wout_sb, wout)]
                else:
                    first_extra = []

                def proj(qb, rep=rep, extra=None):
                    qsl = slice(qb * QB, (qb + 1) * QB)
                    xc = [xpool.tile([P, NCC, QB], bf16, tag=f"xc{b}",
                                     name=f"xc{b}_{rep}_{qb}")
                          for b in range(B)]
                    for o in range(NCC):  # split so compute starts early
                        nc.sync.dma_start(xc[0][:, o, :], xt_r[0][:, o, qsl])
                    if extra:
                        for sb, dr in extra:
                            nc.sync.dma_start(
                                sb[:], dr.rearrange("(o p) m -> p o m", p=P))
                        nc.sync.dma_start(bout_row[:], bout)
                        nc.gpsimd.partition_broadcast(
                            bias_rep.rearrange("p h x -> p (h x)"),
                            bout_row[:], channels=P)
                    for o in range(NCC):
                        nc.sync.dma_start(xc[1][:, o, :], xt_r[1][:, o, qsl])
                    # batch-0 q/k first so its attention unblocks early;
                    # slab writes via ACT (idle-ish here) so they aren't
                    # queued behind DVE mask/normalize work
                    proj_groups = [
                        (wq_sb, 0, 0, qt[0]), (wk_sb, 1, 0, kt[0]),
                        (wq_sb, 0, 1, qt[1]), (wk_sb, 1, 1, kt[1]),
                    ]
                    for gi, (wsb, bi, b, dest) in enumerate(proj_groups[:2]):
                        ps = psum.tile([P, QB], f32, tag="a",
                                       name=f"proj_{rep}_{qb}_{gi}")
                        for o in range(NCC):
                            nc.tensor.matmul(
                                ps[:], wsb[:, o, :], xc[b][:, o, :],
                                start=(o == 0), stop=(o == NCC - 1),
                            )
                        nc.scalar.activation(dest[:, qsl], ps[:], Ident,
                                             bias=bqk_sb[:, bi:bi + 1])
                    for b in range(B):
                        for tt in range(QB // P):
                            t0 = qb * (QB // P) + tt
                            pv = psum.tile([P, P], f32, tag="yt", bufs=4,
                                           name=f"pv_{rep}_{b}_{t0}")
                            for o in range(NCC):
                                nc.tensor.matmul(
                                    pv[:], xc[b][:, o, tt * P:(tt + 1) * P],
                                    wv_sb[:, o, :],
                                    start=(o == 0), stop=(o == NCC - 1),
                                )
                            nc.vector.tensor_copy(
                                out=vsl[b][:, t0, :, 0:HD],
                                in_=pv.rearrange("p (h x) -> p h x", h=HPC))
                    for gi, (wsb, bi, b, dest) in enumerate(proj_groups[2:]):
                        ps = psum.tile([P, QB], f32, tag="a",
                                       name=f"proj_{rep}_{qb}_{2 + gi}")
                        for o in range(NCC):
                            nc.tensor.matmul(
                                ps[:], wsb[:, o, :], xc[b][:, o, :],
                                start=(o == 0), stop=(o == NCC - 1),
                            )
                        nc.vector.tensor_scalar_add(dest[:, qsl], ps[:],
                                                    bqk_sb[:, bi:bi + 1])

                def attn(b, qb, rep=rep):
                    qsl0 = qb * QB
                    ytp = [psum.tile([P, QB], f32, tag="yt", bufs=4,
                                     name=f"yt_{rep}_{b}_{qb}_{h}")
                           for h in range(2)]
                    nkc = (qb + 1) * (QB // P)
                    DEPTH = 2  # av lags scores by 2 chunks so the PE never
                    # waits on the ACT exp / DVE mask of the current chunk
                    ets = {}

                    def av(kc):
                        off = kc * P - qb * QB
                        es = off if off >= 0 else 0
                        first, last = kc == 0, kc == nkc - 1
                        for h in range(2):
                            # [v | 1] lhsT: row 64 of the output is Z
                            nc.tensor.matmul(
                                ytp[h][0:HD + 1, es:],
                                vsl[b][:, kc, h, :],
                                ets[kc][:, h, es:],
                                start=first, stop=last,
                                skip_group_check=True,
                            )

                    for kc in range(nkc):
                        off = kc * P - qb * QB
                        diag = off >= 0
                        es = off if diag else 0
                        sc = psum.tile([P, 2, QB], f32, tag="a",
                                       name=f"sc_{rep}_{b}_{qb}_{kc}")
                        for h in range(2):
                            hp = slice(h * 64, (h + 1) * 64)
                            nc.tensor.matmul(
                                sc[:, h, es:],
                                kt[b][hp, kc * P:(kc + 1) * P],
                                qt[b][hp, qsl0 + es:qsl0 + QB],
                                start=True, stop=True,
                                tile_position=(h * 64, 0),
                                skip_group_check=True,
                            )
                        et = work.tile([P, 2, QB], bf16, tag="et", bufs=4,
                                       name=f"et_{rep}_{b}_{qb}_{kc}")
                        ets[kc] = et
                        nc.scalar.activation(et[:, :, es:], sc[:, :, es:],
                                             Exp, scale=0.125)
                        if diag:  # zero the non-causal band
                            be = min(off + P, QB)
                            for h in range(2):
                                nc.vector.tensor_tensor(
                                    et[:, h, es:be], et[:, h, es:be],
                                    maskS[:, 384 - off + es:384 - off + be],
                                    mult)
                        if kc >= DEPTH:
                            av(kc - DEPTH)
                    for kc in range(max(0, nkc - DEPTH), nkc):
                        av(kc)
                    for h in range(2):
                        # 1/Z replicated across the 64 head dims via a K=1
                        # ones-matmul — NOT gpsimd.partition_broadcast: the
                        # collective_compute instruction blocks the gpsimd
                        # queue until the ring completes, which would stall
                        # every later broadcast behind an AllToAll
                        zi = work.tile([1, QB], bf16, tag="zi",
                                       name=f"zi_{rep}_{b}_{qb}_{h}")
                        with nc.allow_low_precision(
                                reason="bf16 1/Z feeds replicate matmul"):
                            nc.vector.reciprocal(zi[:], ytp[h][HD:HD + 1, :])
                        zr = psum.tile([HD, QB], f32, tag="yt", bufs=4,
                                       name=f"zr_{rep}_{b}_{qb}_{h}")
                        nc.tensor.matmul(zr[:], ones1_64[:], zi[:],
                                         start=True, stop=True,
                                         skip_group_check=True)
                        zrs = work.tile([HD, QB], f32, tag="zrs",
                                        name=f"zrs_{rep}_{b}_{qb}_{h}")
                        nc.vector.tensor_copy(out=zrs[:], in_=zr[:])
                        nc.vector.tensor_tensor(
                            ytsb[b][h * HD:(h + 1) * HD, qsl0:qsl0 + QB],
                            ytp[h][0:HD, :], zrs[:], mult)

                # out-projection for one gathered token stripe: full 1024
                # columns, contraction over the 8 head-chunks (one per rank)
                def outproj(qb, ytg, rep=rep):
                    ytt = work.tile([P, N_CORES, P], bf16, tag="ytt", bufs=4,
                                    name=f"ytt_{rep}_{qb}")
                    nc.sync.dma_start(ytt[:],
                                      ytg.rearrange("i h t -> h i t"))
                    osb = work.tile([P, 2, QB], f32, tag="osb", bufs=2,
                                    name=f"osb_{rep}_{qb}")
                    for half in range(2):
                        hsl = slice(half * QB, (half + 1) * QB)
                        po = psum.tile([P, QB], f32, tag="a",
                                       name=f"po_{rep}_{qb}_{half}")
                        for i in range(N_CORES):
                            nc.tensor.matmul(
                                po[:], ytt[:, i, :], wout_sb[:, i, hsl],
                                start=(i == 0), stop=(i == N_CORES - 1),
                                skip_group_check=True,
                            )
                        nc.vector.tensor_tensor(osb[:, half, :], po[:],
                                                bias_rep[:, half, :], add)
                    nc.sync.dma_start(
                        out[qb].rearrange("p (h x) -> p h x", h=2), osb[:])

                # phase A: all projections back-to-back (dense PE, HAM warm)
                for qb in range(NQB):
                    proj(qb, extra=(first_extra if qb == 0 else None))
                # phase B: attention in DESCENDING block order — the big
                # causal blocks finish (and start their AllToAll) early,
                # and the accumulated out-projections cover the small
                # final block's exchange latency
                ytgs = {}
                order = list(range(NQB - 1, -1, -1))
                for oi, qb in enumerate(order):
                    qsl = slice(qb * QB, (qb + 1) * QB)
                    attn(0, qb)
                    attn(1, qb)
                    # flush y^T (bf16) and exchange: shard j = my 2 heads,
                    # tokens of (batch j//4, stripe j%4) within this block
                    ytl = dram.tile([N_CORES, P, P], bf16,
                                    name=f"ytl_{rep}_{qb}")
                    for b in range(B):
                        nc.sync.dma_start(
                            ytl[b * 4:(b + 1) * 4].rearrange("j h t -> h j t"),
                            ytsb[b][:, qsl].rearrange("h (j t) -> h j t", j=4))
                    ytg = dram.tile([N_CORES, P, P], bf16,
                                    name=f"ytg_{rep}_{qb}")
                    if single:
                        for g in range(N_CORES):  # timing stand-in
                            nc.gpsimd.dma_start(ytg[g], ytl[g])
                    else:
                        nc.gpsimd.collective_compute(
                            "AllToAll",
                            mybir.AluOpType.bypass,
                            replica_groups=[list(range(N_CORES))],
                            ins=[ytl.opt()],
                            outs=[ytg.opt()],
                        )
                    ytgs[qb] = ytg
                    if dbg and rep == 0:
                        for b in range(B):
                            nc.sync.dma_start(ydbg[b][:, qsl],
                                              ytsb[b][:, qsl])
                # all out-projections run at the tail: blocks 3..1 exchanged
                # long ago, so their matmuls fill the PE while block 0's
                # AllToAll completes
                for qb in order:
                    outproj(qb, ytgs[qb])
                if dbg and rep == 0:
                    for qb in range(NQB):
                        gsb = work.tile([P, N_CORES, P], bf16, tag="ytt",
                                        bufs=2, name=f"gsb_{qb}")
                        nc.sync.dma_start(
                            gsb[:], ytgs[qb].rearrange("i h t -> h i t"))
                        nc.sync.dma_start(
                            gdbg[qb].rearrange("i h t -> h i t"), gsb[:])

    nc.compile()
    return nc


def _get_program(repeat: int = 1, single: bool = False, dbg: bool = False):
    key = ("nc", repeat, single, dbg)
    if key not in _cache:
        _cache[key] = _build_program(repeat, single, dbg)
    return _cache[key]


def prepare_in_maps(x, w_qkv, b_qkv, w_out, b_out):
    """Shard full inputs into the 8 per-core input maps."""
    import concourse.mybir as mybir

    np_bf16 = mybir.dt.np(mybir.dt.bfloat16)

    x = np.asarray(x, dtype=np.float32)
    w_qkv = np.asarray(w_qkv, dtype=np.float32)
    b_qkv = np.asarray(b_qkv, dtype=np.float32)
    w_out = np.asarray(w_out, dtype=np.float32)
    b_out = np.asarray(b_out, dtype=np.float32)

    xts = [np.ascontiguousarray(x[b].T).astype(np_bf16) for b in range(B)]
    # softmax rows sum to 1 => y = attn @ v + b_v exactly, so the v-bias
    # folds into an effective output bias on the host
    b_out_eff = (b_out.astype(np.float64)
                 + b_qkv[2 * C:].astype(np.float64) @ w_out.astype(np.float64)
                 ).astype(np.float32)
    wout_bf = np.ascontiguousarray(w_out).astype(np_bf16)

    in_maps = []
    for c in range(N_CORES):
        qc = slice(c * HPC * HD, (c + 1) * HPC * HD)  # 128 cols, my 2 heads
        in_maps.append({
            "xt0": xts[0],
            "xt1": xts[1],
            "wq": np.ascontiguousarray(w_qkv[:, qc]).astype(np_bf16),
            "wk": np.ascontiguousarray(
                w_qkv[:, C + qc.start:C + qc.stop]).astype(np_bf16),
            "wv": np.ascontiguousarray(
                w_qkv[:, 2 * C + qc.start:2 * C + qc.stop]).astype(np_bf16),
            "wout": wout_bf,
            "bqk": np.ascontiguousarray(
                np.stack([b_qkv[qc], b_qkv[C + qc.start:C + qc.stop]])),
            "bout": np.ascontiguousarray(b_out_eff[None, :]),
        })
    return in_maps


def run_device(in_maps, repeat: int = 1):
    """Execute the compiled SPMD program; returns per-core result dicts.

    The NeuronCores occasionally come up wedged (NRT_EXEC_UNIT_UNRECOVERABLE
    / LoadExecutable failures) if a previous process died mid-execution;
    they recover after a short wait, so retry with backoff.
    """
    import time as _time
    from concourse import bass_utils

    nc = _get_program(repeat)
    last_err = None
    for attempt in range(3):
        try:
            res = bass_utils.run_bass_kernel_spmd(
                nc, in_maps, core_ids=list(range(N_CORES)))
            return res.results
        except Exception as e:  # device wedge: wait for recovery and retry
            last_err = e
            if attempt < 2:
                _time.sleep(75)
    raise last_err


def assemble_output(results):
    out = np.empty((B, T, C), dtype=np.float32)
    for c in range(N_CORES):
        b, j = divmod(c, 4)
        o = results[c]["out"]  # [NQB, 128, C]
        for qb in range(NQB):
            t0 = qb * QB + j * P
            out[b, t0:t0 + P, :] = o[qb]
    return out


def kernel(x, w_qkv, b_qkv, w_out, b_out):
    in_maps = prepare_in_maps(x, w_qkv, b_qkv, w_out, b_out)
    results = run_device(in_maps)
    return assemble_output(results)


if __name__ == "__main__":
    rng = np.random.default_rng(0)
    inputs = {
        "x": rng.standard_normal((B, T, C), dtype=np.float32),
        "w_qkv": rng.standard_normal((C, 3 * C), dtype=np.float32) / np.sqrt(C),
        "b_qkv": rng.standard_normal(3 * C, dtype=np.float32) * 0.1,
        "w_out": rng.standard_normal((C, C), dtype=np.float32) / np.sqrt(C),
        "b_out": rng.standard_normal(C, dtype=np.float32) * 0.1,
    }
    y = kernel(**inputs)
    print("kernel output:", y.shape, y.dtype, float(np.abs(y).max()))


# revision 18
# speedup vs baseline: 1.0922x; 1.0207x over previous
"""Self-contained Trainium2 Bass kernel for causal multi-head attention.

Problem: y = Attention(x) with B=2, T=2048, C=1024, H=16 heads, HD=64,
causal softmax, fused qkv projection and output projection.

Sharding (8 NeuronCores): core c owns heads {2c, 2c+1} for BOTH batches
(tensor parallel on heads; the batch dim rides along inside each core as
two attention "pairs"). All compute runs in bf16 (fp32 PSUM accumulation):

  per token-block qb (512 tokens, 4 blocks):
    qkv projection for both batches (q/k transposed: head-dim on
    partitions; v natural; softmax normalizer Z rides as a 65th ones
    column on v)
    causal-softmax attention for batch 0 then batch 1 (two heads
    row-tiled concurrently on the PE)
    normalized y^T (bf16) is flushed to DRAM and exchanged with a
    single 8-way AllToAll: shard j of block qb = y^T[my 2 heads,
    tokens of (batch j//4, stripe j%4)] -> after the exchange each
    core holds ALL 16 heads for its own 128-token stripe of qb
    out-projection for block qb-1 (full 1024 columns, contraction over
    the 8 gathered head-chunks) overlaps the next block's attention

The AllToAll moves only ~0.25 MB/core total (vs 6 MB for a y AllGather),
so all communication hides behind attention compute. The v-bias is folded
into an effective output bias on the host (softmax rows sum to 1).
"""

import sys

sys.path.insert(0, "/opt/trn_rl_repo")

import numpy as np

B, T, C, H, HD = 2, 2048, 1024, 16, 64
P = 128
N_CORES = 8
HPC = 2  # heads per core
QB = 512  # query block (free dim of attention matmuls)
NQB = T // QB  # 4
NCC = C // P  # 8 contraction chunks
NTT = T // P  # 16 token tiles

_cache: dict = {}


def _build_program(repeat: int = 1, single: bool = False, dbg: bool = False):
    """Build + compile the per-core Bass program (same program on all cores)."""
    import concourse.bass as bass
    import concourse.mybir as mybir
    from concourse import bacc, tile

    f32 = mybir.dt.float32
    f32r = mybir.dt.float32r
    bf16 = mybir.dt.bfloat16
    Exp = mybir.ActivationFunctionType.Exp
    Ident = mybir.ActivationFunctionType.Identity
    mult = mybir.AluOpType.mult
    add = mybir.AluOpType.add

    nc = bacc.Bacc("TRN2", target_bir_lowering=False, debug=False,
                   num_devices=N_CORES)

    xt0 = nc.dram_tensor("xt0", [C, T], bf16, kind="ExternalInput").ap()
    xt1 = nc.dram_tensor("xt1", [C, T], bf16, kind="ExternalInput").ap()
    wq = nc.dram_tensor("wq", [C, P], bf16, kind="ExternalInput").ap()
    wk = nc.dram_tensor("wk", [C, P], bf16, kind="ExternalInput").ap()
    wv = nc.dram_tensor("wv", [C, P], bf16, kind="ExternalInput").ap()
    wout = nc.dram_tensor("wout", [C, C], bf16, kind="ExternalInput").ap()
    bqk = nc.dram_tensor("bqk", [2, P], f32, kind="ExternalInput").ap()
    bout = nc.dram_tensor("bout", [1, C], f32, kind="ExternalInput").ap()
    out = nc.dram_tensor("out", [NQB, P, C], f32, kind="ExternalOutput").ap()
    if dbg:
        ydbg = nc.dram_tensor("ydbg", [B, P, T], bf16,
                              kind="ExternalOutput").ap()
        gdbg = nc.dram_tensor("gdbg", [NQB, N_CORES, P, P], bf16,
                              kind="ExternalOutput").ap()

    xt_r = [xt0.rearrange("(o p) t -> p o t", p=P),
            xt1.rearrange("(o p) t -> p o t", p=P)]

    with tile.TileContext(nc) as tc:
        import contextlib

        with contextlib.ExitStack() as ctx:
            const = ctx.enter_context(tc.tile_pool(name="const", bufs=1))
            wpool = ctx.enter_context(tc.tile_pool(name="wpool", bufs=1))
            xpool = ctx.enter_context(tc.tile_pool(name="xpool", bufs=2))
            slab = ctx.enter_context(tc.tile_pool(name="slab", bufs=1))
            work = ctx.enter_context(tc.tile_pool(name="work", bufs=3))
            psum = ctx.enter_context(tc.tile_pool(name="psum", bufs=2,
                                                  space="PSUM"))
            dram = ctx.enter_context(tc.tile_pool(name="dram", bufs=1,
                                                  space="DRAM"))

            # ---- constants ----
            ones1_64 = const.tile([1, 64], bf16)
            nc.vector.memset(ones1_64[:], 1.0)
            ones1 = const.tile([1, P], bf16)
            nc.vector.memset(ones1[:], 1.0)
            # causal mask strip: maskS[kp, u] = 1.0 iff u - kp - 384 >= 0.
            # chunk (qb, kc) with off = kc*128 - qb*512 in {0,128,256,384}
            # uses slice maskS[:, 384-off+c] for chunk column c.
            maskF = const.tile([P, 896], f32)
            nc.vector.memset(maskF[:], 1.0)
            nc.gpsimd.affine_select(
                out=maskF[:],
                in_=maskF[:],
                compare_op=mybir.AluOpType.is_ge,
                fill=0.0,
                base=-384,
                pattern=[[1, 896]],
                channel_multiplier=-1,
            )
            maskS = const.tile([P, 896], bf16)
            nc.vector.tensor_copy(out=maskS[:], in_=maskF[:])
            bqk_sb = const.tile([P, 2], f32)
            bout_row = const.tile([1, C], f32)
            bias_rep = const.tile([P, 2, QB], f32)

            # ---- weights (w_out DMA is issued later, behind the first x) ----
            wq_sb = wpool.tile([P, NCC, P], bf16, name="w_q")
            wk_sb = wpool.tile([P, NCC, P], bf16, name="w_k")
            wv_sb = wpool.tile([P, NCC, P], bf16, name="w_v")
            wout_sb = wpool.tile([P, NCC, C], bf16, name="w_out")
            nc.sync.dma_start(wq_sb[:], wq.rearrange("(o p) m -> p o m", p=P))
            nc.sync.dma_start(bqk_sb[:], bqk.rearrange("g p -> p g"))

            # PE warm-up: dependency-free matmuls on constants run during
            # the initial DMA-only window so the HAM clock gate reaches
            # full rate before the first real matmul
            warm = psum.tile([64, P], f32, tag="sm", bufs=2, name="warm")
            for _ in range(32):
                nc.tensor.matmul(warm[:], ones1_64[:], ones1[:],
                                 start=True, stop=True,
                                 skip_group_check=True)

            # ---- persistent slabs (index = batch) ----
            qt = [slab.tile([P, T], bf16, name=f"qt{b}") for b in range(B)]
            kt = [slab.tile([P, T], bf16, name=f"kt{b}") for b in range(B)]
            # v slab: per token-tile, 2 head slots of [64 v-dims | 1.0]
            # (the ones column folds the softmax normalizer Z into attn @ v)
            vsl = [slab.tile([P, NTT, HPC, HD + 1], bf16, name=f"vsl{b}")
                   for b in range(B)]
            for b in range(B):
                nc.vector.memset(
                    vsl[b].rearrange("p t h x -> p (t h) x")[:, :, HD:HD + 1],
                    1.0)
            ytsb = [slab.tile([P, T], bf16, name=f"ytsb{b}") for b in range(B)]

            for rep in range(repeat):
                if rep == 0:
                    first_extra = [
                        (wk_sb, wk), (wv_sb, wv), (wout_sb, wout)]
                else:
                    first_extra = []

                # The PE queue is strict FIFO, so emission order IS the PE
                # schedule. Attention is ACT-bound (the exp of each chunk is
                # slower than its matmuls), so projection / out-projection
                # matmuls are emitted as "filler" closures BETWEEN attention
                # chunks — the PE stays dense (and the HAM clock stays warm)
                # while ACT works through the exps.
                def proj_pieces(qb, rep=rep, extra=None):
                    """Emit block qb's x DMAs now; return compute closures."""
                    qsl = slice(qb * QB, (qb + 1) * QB)
                    xc = [xpool.tile([P, NCC, QB], bf16, tag=f"xc{b}",
                                     name=f"xc{b}_{rep}_{qb}")
                          for b in range(B)]
                    for o in range(NCC):  # split so compute starts early
                        nc.sync.dma_start(xc[0][:, o, :], xt_r[0][:, o, qsl])
                    if extra:
                        for sb, dr in extra:
                            nc.sync.dma_start(
                                sb[:], dr.rearrange("(o p) m -> p o m", p=P))
                        nc.sync.dma_start(bout_row[:], bout)
                        nc.gpsimd.partition_broadcast(
                            bias_rep.rearrange("p h x -> p (h x)"),
                            bout_row[:], channels=P)
                    for o in range(NCC):
                        nc.sync.dma_start(xc[1][:, o, :], xt_r[1][:, o, qsl])

                    def qk(wsb, bi, b, dest, gi):
                        def run():
                            ps = psum.tile([P, QB], f32, tag="sm", bufs=2,
                                           name=f"proj_{rep}_{qb}_{gi}")
                            for o in range(NCC):
                                nc.tensor.matmul(
                                    ps[:], wsb[:, o, :], xc[b][:, o, :],
                                    start=(o == 0), stop=(o == NCC - 1),
                                )
                            if gi < 2:  # ACT for b0, DVE for b1 (balance)
                                nc.scalar.activation(
                                    dest[:, qsl], ps[:], Ident,
                                    bias=bqk_sb[:, bi:bi + 1])
                            else:
                                nc.vector.tensor_scalar_add(
                                    dest[:, qsl], ps[:],
                                    bqk_sb[:, bi:bi + 1])
                        return run

                    def vtile(b, tt):
                        def run():
                            t0 = qb * (QB // P) + tt
                            pv = psum.tile([P, P], f32, tag="sm", bufs=2,
                                           name=f"pv_{rep}_{b}_{t0}")
                            for o in range(NCC):
                                nc.tensor.matmul(
                                    pv[:], xc[b][:, o, tt * P:(tt + 1) * P],
                                    wv_sb[:, o, :],
                                    start=(o == 0), stop=(o == NCC - 1),
                                )
                            nc.vector.tensor_copy(
                                out=vsl[b][:, t0, :, 0:HD],
                                in_=pv.rearrange("p (h x) -> p h x", h=HPC))
                        return run

                    pieces = [qk(wq_sb, 0, 0, qt[0], 0),
                              qk(wk_sb, 1, 0, kt[0], 1)]
                    pieces += [vtile(0, tt) for tt in range(QB // P)]
                    pieces += [qk(wq_sb, 0, 1, qt[1], 2),
                               qk(wk_sb, 1, 1, kt[1], 3)]
                    pieces += [vtile(1, tt) for tt in range(QB // P)]
                    return pieces

                def attn_core(b, qb, fillers, rep=rep):
                    qsl0 = qb * QB
                    ytp = [psum.tile([P, QB], f32, tag="yt", bufs=2,
                                     name=f"yt_{rep}_{b}_{qb}_{h}")
                           for h in range(2)]
                    nkc = (qb + 1) * (QB // P)
                    DEPTH = 2  # av lags scores so the PE never waits on the
                    # current chunk's ACT exp / DVE mask
                    ets = {}

                    def av(kc):
                        off = kc * P - qb * QB
                        es = off if off >= 0 else 0
                        first, last = kc == 0, kc == nkc - 1
                        for h in range(2):
                            # [v | 1] lhsT: row 64 of the output is Z
                            nc.tensor.matmul(
                                ytp[h][0:HD + 1, es:],
                                vsl[b][:, kc, h, :],
                                ets[kc][:, h, es:],
                                start=first, stop=last,
                                skip_group_check=True,
                            )

                    for kc in range(nkc):
                        off = kc * P - qb * QB
                        diag = off >= 0
                        es = off if diag else 0
                        sc = psum.tile([P, 2, QB], f32, tag="a",
                                       name=f"sc_{rep}_{b}_{qb}_{kc}")
                        for h in range(2):
                            hp = slice(h * 64, (h + 1) * 64)
                            nc.tensor.matmul(
                                sc[:, h, es:],
                                kt[b][hp, kc * P:(kc + 1) * P],
                                qt[b][hp, qsl0 + es:qsl0 + QB],
                                start=True, stop=True,
                                tile_position=(h * 64, 0),
                                skip_group_check=True,
                            )
                        et = work.tile([P, 2, QB], bf16, tag="et", bufs=4,
                                       name=f"et_{rep}_{b}_{qb}_{kc}")
                        ets[kc] = et
                        nc.scalar.activation(et[:, :, es:], sc[:, :, es:],
                                             Exp, scale=0.125)
                        if diag:  # zero the non-causal band
                            be = min(off + P, QB)
                            for h in range(2):
                                nc.vector.tensor_tensor(
                                    et[:, h, es:be], et[:, h, es:be],
                                    maskS[:, 384 - off + es:384 - off + be],
                                    mult)
                        if kc >= DEPTH:
                            av(kc - DEPTH)
                        if fillers:
                            fillers.pop(0)()
                    for kc in range(max(0, nkc - DEPTH), nkc):
                        av(kc)
                    return ytp

                def normalize(b, qb, ytp, rep=rep):
                    qsl0 = qb * QB
                    for h in range(2):
                        # 1/Z replicated across the 64 head dims via a K=1
                        # ones-matmul — NOT gpsimd.partition_broadcast: the
                        # collective_compute instruction blocks the gpsimd
                        # queue until its ring completes, which would stall
                        # every later broadcast behind an AllToAll
                        zi = work.tile([1, QB], bf16, tag="zi",
                                       name=f"zi_{rep}_{b}_{qb}_{h}")
                        with nc.allow_low_precision(
                                reason="bf16 1/Z feeds replicate matmul"):
                            nc.vector.reciprocal(zi[:], ytp[h][HD:HD + 1, :])
                        zr = psum.tile([HD, QB], f32, tag="sm", bufs=2,
                                       name=f"zr_{rep}_{b}_{qb}_{h}")
                        nc.tensor.matmul(zr[:], ones1_64[:], zi[:],
                                         start=True, stop=True,
                                         skip_group_check=True)
                        zrs = work.tile([HD, QB], f32, tag="zrs",
                                        name=f"zrs_{rep}_{b}_{qb}_{h}")
                        nc.vector.tensor_copy(out=zrs[:], in_=zr[:])
                        nc.vector.tensor_tensor(
                            ytsb[b][h * HD:(h + 1) * HD, qsl0:qsl0 + QB],
                            ytp[h][0:HD, :], zrs[:], mult)

                # out-projection for one gathered token stripe: full 1024
                # columns, contraction over the 8 head-chunks (one per rank)
                def outproj_pieces(qb, ytg, rep=rep):
                    ytt = work.tile([P, N_CORES, P], bf16, tag="ytt", bufs=4,
                                    name=f"ytt_{rep}_{qb}")
                    nc.sync.dma_start(ytt[:],
                                      ytg.rearrange("i h t -> h i t"))
                    osb = work.tile([P, 2, QB], f32, tag="osb", bufs=2,
                                    name=f"osb_{rep}_{qb}")
                    done = []

                    def half(hf):
                        def run():
                            hsl = slice(hf * QB, (hf + 1) * QB)
                            po = psum.tile([P, QB], f32, tag="sm", bufs=2,
                                           name=f"po_{rep}_{qb}_{hf}")
                            for i in range(N_CORES):
                                nc.tensor.matmul(
                                    po[:], ytt[:, i, :], wout_sb[:, i, hsl],
                                    start=(i == 0),
                                    stop=(i == N_CORES - 1),
                                    skip_group_check=True,
                                )
                            nc.vector.tensor_tensor(osb[:, hf, :], po[:],
                                                    bias_rep[:, hf, :], add)
                            done.append(hf)
                            if len(done) == 2:
                                nc.sync.dma_start(
                                    out[qb].rearrange("p (h x) -> p h x",
                                                      h=2), osb[:])
                        return run

                    return [half(0), half(1)]

                # block 0's projection runs up front (nothing to hide it in)
                for pc in proj_pieces(0, extra=first_extra):
                    pc()
                ytgs = {}
                fillers = []
                for qb in range(NQB):
                    qsl = slice(qb * QB, (qb + 1) * QB)
                    if qb + 1 < NQB:
                        fillers += proj_pieces(qb + 1)
                    if qb - 1 >= 0:
                        fillers += outproj_pieces(qb - 1, ytgs[qb - 1])
                    ytp0 = attn_core(0, qb, fillers)
                    normalize(0, qb, ytp0)
                    ytp1 = attn_core(1, qb, fillers)
                    while fillers:  # PE works these while DVE normalizes
                        fillers.pop(0)()
                    normalize(1, qb, ytp1)
                    # flush y^T (bf16) and exchange: shard j = my 2 heads,
                    # tokens of (batch j//4, stripe j%4) within this block
                    ytl = dram.tile([N_CORES, P, P], bf16,
                                    name=f"ytl_{rep}_{qb}")
                    for b in range(B):
                        nc.sync.dma_start(
                            ytl[b * 4:(b + 1) * 4].rearrange("j h t -> h j t"),
                            ytsb[b][:, qsl].rearrange("h (j t) -> h j t", j=4))
                    ytg = dram.tile([N_CORES, P, P], bf16,
                                    name=f"ytg_{rep}_{qb}")
                    if single:
                        for g in range(N_CORES):  # timing stand-in
                            nc.gpsimd.dma_start(ytg[g], ytl[g])
                    else:
                        nc.gpsimd.collective_compute(
                            "AllToAll",
                            mybir.AluOpType.bypass,
                            replica_groups=[list(range(N_CORES))],
                            ins=[ytl.opt()],
                            outs=[ytg.opt()],
                        )
                    ytgs[qb] = ytg
                    if dbg and rep == 0:
                        for b in range(B):
                            nc.sync.dma_start(ydbg[b][:, qsl],
                                              ytsb[b][:, qsl])
                # tail: only the last block's out-projection remains
                for pc in outproj_pieces(NQB - 1, ytgs[NQB - 1]):
                    pc()
                if dbg and rep == 0:
                    for qb in range(NQB):
                        gsb = work.tile([P, N_CORES, P], bf16, tag="ytt",
                                        bufs=2, name=f"gsb_{qb}")
                        nc.sync.dma_start(
                            gsb[:], ytgs[qb].rearrange("i h t -> h i t"))
                        nc.sync.dma_start(
                            gdbg[qb].rearrange("i h t -> h i t"), gsb[:])

    nc.compile()
    return nc


def _get_program(repeat: int = 1, single: bool = False, dbg: bool = False):
    key = ("nc", repeat, single, dbg)
    if key not in _cache:
        _cache[key] = _build_program(repeat, single, dbg)
    return _cache[key]


def prepare_in_maps(x, w_qkv, b_qkv, w_out, b_out):
    """Shard full inputs into the 8 per-core input maps."""
    import concourse.mybir as mybir

    np_bf16 = mybir.dt.np(mybir.dt.bfloat16)

    x = np.asarray(x, dtype=np.float32)
    w_qkv = np.asarray(w_qkv, dtype=np.float32)
    b_qkv = np.asarray(b_qkv, dtype=np.float32)
    w_out = np.asarray(w_out, dtype=np.float32)
    b_out = np.asarray(b_out, dtype=np.float32)

    xts = [np.ascontiguousarray(x[b].T).astype(np_bf16) for b in range(B)]
    # softmax rows sum to 1 => y = attn @ v + b_v exactly, so the v-bias
    # folds into an effective output bias on the host
    b_out_eff = (b_out.astype(np.float64)
                 + b_qkv[2 * C:].astype(np.float64) @ w_out.astype(np.float64)
                 ).astype(np.float32)
    wout_bf = np.ascontiguousarray(w_out).astype(np_bf16)

    in_maps = []
    for c in range(N_CORES):
        qc = slice(c * HPC * HD, (c + 1) * HPC * HD)  # 128 cols, my 2 heads
        in_maps.append({
            "xt0": xts[0],
            "xt1": xts[1],
            "wq": np.ascontiguousarray(w_qkv[:, qc]).astype(np_bf16),
            "wk": np.ascontiguousarray(
                w_qkv[:, C + qc.start:C + qc.stop]).astype(np_bf16),
            "wv": np.ascontiguousarray(
                w_qkv[:, 2 * C + qc.start:2 * C + qc.stop]).astype(np_bf16),
            "wout": wout_bf,
            "bqk": np.ascontiguousarray(
                np.stack([b_qkv[qc], b_qkv[C + qc.start:C + qc.stop]])),
            "bout": np.ascontiguousarray(b_out_eff[None, :]),
        })
    return in_maps


def run_device(in_maps, repeat: int = 1):
    """Execute the compiled SPMD program; returns per-core result dicts.

    The NeuronCores occasionally come up wedged (NRT_EXEC_UNIT_UNRECOVERABLE
    / LoadExecutable failures) if a previous process died mid-execution;
    they recover after a short wait, so retry with backoff.
    """
    import time as _time
    from concourse import bass_utils

    nc = _get_program(repeat)
    last_err = None
    for attempt in range(3):
        try:
            res = bass_utils.run_bass_kernel_spmd(
                nc, in_maps, core_ids=list(range(N_CORES)))
            return res.results
        except Exception as e:  # device wedge: wait for recovery and retry
            last_err = e
            if attempt < 2:
                _time.sleep(75)
    raise last_err


def assemble_output(results):
    out = np.empty((B, T, C), dtype=np.float32)
    for c in range(N_CORES):
        b, j = divmod(c, 4)
        o = results[c]["out"]  # [NQB, 128, C]
        for qb in range(NQB):
            t0 = qb * QB + j * P
            out[b, t0:t0 + P, :] = o[qb]
    return out


def kernel(x, w_qkv, b_qkv, w_out, b_out):
    in_maps = prepare_in_maps(x, w_qkv, b_qkv, w_out, b_out)
    results = run_device(in_maps)
    return assemble_output(results)


if __name__ == "__main__":
    rng = np.random.default_rng(0)
    inputs = {
        "x": rng.standard_normal((B, T, C), dtype=np.float32),
        "w_qkv": rng.standard_normal((C, 3 * C), dtype=np.float32) / np.sqrt(C),
        "b_qkv": rng.standard_normal(3 * C, dtype=np.float32) * 0.1,
        "w_out": rng.standard_normal((C, C), dtype=np.float32) / np.sqrt(C),
        "b_out": rng.standard_normal(C, dtype=np.float32) * 0.1,
    }
    y = kernel(**inputs)
    print("kernel output:", y.shape, y.dtype, float(np.abs(y).max()))
